# revision 1
# baseline (speedup 1.0000x reference)
"""Trainium2 Bass kernel: 1024-point FFT of real rows -> (real, imag).

Math: out = FFT_1024(x[b, :]) per row. Two folding levels over the real
input x (U[n] = x[n]+x[1024-n], V[n] = x[n]-x[1024-n]), then a radix-2
split of the half-spectrum k in [1,513) by parity:
  Xr[2m]   = sum_{n<256} Aue[n] cos(2pi n m/512)   + U[256](-1)^m
  Xi[2m]   = sum_{n<256} Avo[n] (-sin(2pi n m/512))
  Xr[2m+1] = sum_{n<256} Auo[n] cos(pi n(2m+1)/512)
  Xi[2m+1] = sum_{n<256} Ave[n] (-sin(pi n(2m+1)/512)) - V[256](-1)^m
with Aue/Auo/Avo/Ave the second-level even/odd folds of U and V. The
remaining half follows from X[1024-k] = conj(X[k]); k=0 is a row sum.

The device computes the four quadrants in TRANSPOSED orientation (freq
on PSUM partitions, batch on the free dim): per 512-row group and
128-wide k-tile, K=256 fp32r matmuls (2 accumulating chunks) with the
quarter-size cos/sin matrices as the stationary operand — 18 matmuls
per group. Every rank-1 edge term rides a dead coefficient row: row 0
of Aue/Auo absorbs U[0] +/- x[512] (all-ones coefficient rows), row 0
of Avo carries U[256] (killed by Cei's zero row, applied to the
even-real quadrant by one extra matmul against the one-hot `alt` row),
and row 0 of Ave carries V[256] via Coi's overridden row 0.

The host ships the four folded arrays (same total bytes as x,
group-blocked so every DMA moves one long contiguous run per partition)
and performs the pure data-expansion assembly: parity interleave,
conjugate mirror, k=0 column, final layout transpose. Inputs ride the
sync queue, real/imag outputs the gpsimd/scalar queues. Pure
data-parallel across 8 cores, no collectives.
"""

import os
import numpy as np

N_FFT = 1024
BATCH = 16384
N_CORES = 8
B_CORE = BATCH // N_CORES  # 2048
P = 128
HALF = 512
QU = 256
GC = 512                   # batch rows per group

_BUILD_CACHE = {}


def _constants():
    n = np.arange(QU, dtype=np.float64)[:, None]
    c = np.arange(QU, dtype=np.float64)[None, :]
    cer = np.cos(2 * np.pi * n * (c + 1) / 512)       # row 0 = 1 (absorbs U0+x512)
    cei = -np.sin(2 * np.pi * n * (c + 1) / 512)      # row 0 = 0 (kills U[256] slot)
    cor = np.cos(np.pi * n * (2 * c + 1) / 512)       # row 0 = 1 (absorbs U0-x512)
    coi = -np.sin(np.pi * n * (2 * c + 1) / 512)
    coi[0, :] = np.where(np.arange(QU) % 2 == 0, -1.0, 1.0)  # carries -V[256](-1)^m
    # one-hot row: (-1)^(p+1) pattern, used by warmup + the ER rank-1 matmul
    alt = np.zeros((P, HALF), dtype=np.float32)
    alt[0, 0::2] = -1.0
    alt[0, 1::2] = 1.0
    f32c = lambda a: np.ascontiguousarray(a.astype(np.float32))
    return f32c(cer), f32c(cei), f32c(cor), f32c(coi), alt


def build_nc(b_core=B_CORE):
    """Build + compile the per-core Bass program (same NEFF on all cores)."""
    import concourse.mybir as mybir
    import concourse.tile as tile
    from concourse import bacc

    f32 = mybir.dt.float32
    f32r = mybir.dt.float32r

    gc = min(GC, b_core)
    n_groups = b_core // gc
    n_kt = QU // P             # 128-wide k-tiles per quadrant (2)

    nc = bacc.Bacc(
        "TRN2", target_bir_lowering=False, debug=False, num_devices=N_CORES
    )

    data_in = {
        name: nc.dram_tensor(name, [n_groups, QU, gc], f32r, kind="ExternalInput")
        for name in ("aue", "auo", "avo", "ave")
    }
    coef_in = {
        name: nc.dram_tensor(name, [QU, QU], f32r, kind="ExternalInput")
        for name in ("cer", "cei", "cor", "coi")
    }
    alt_in = nc.dram_tensor("alt", [P, HALF], f32r, kind="ExternalInput")
    # transposed halves, group-blocked: row r = 4p + slot;
    # slots 0,1 = even-k tiles, 2,3 = odd-k tiles (host interleaves)
    o_rt = nc.dram_tensor("o_rt", [n_groups, HALF, gc], f32, kind="ExternalOutput")
    o_it = nc.dram_tensor("o_it", [n_groups, HALF, gc], f32, kind="ExternalOutput")

    # chunk j / partition p hold row n = 2p+j of the data (and coeff rows)
    data_r = {k: v.ap().rearrange("g (p j) b -> g p j b", j=2)
              for k, v in data_in.items()}
    coef_r = {k: v.ap().rearrange("(p j) k -> p j k", j=2)
              for k, v in coef_in.items()}
    ort_r = o_rt.ap().rearrange("g (p t) b -> g p t b", t=4)
    oit_r = o_it.ap().rearrange("g (p t) b -> g p t b", t=4)

    with tile.TileContext(nc) as tc:
        with (
            tc.tile_pool(name="const", bufs=1) as cpool,
            tc.tile_pool(name="work", bufs=4) as wpool,
            tc.tile_pool(name="outp", bufs=3) as opool,
            tc.tile_pool(name="psm", bufs=4, space="PSUM") as psm,
        ):
            alt_sb = cpool.tile([P, HALF], f32r)
            nc.sync.dma_start(out=alt_sb, in_=alt_in.ap())

            # HAM warmup: keep the PE busy on `alt` while inputs stream in
            # (borrows a "pr" psum slot; it is released untouched)
            wu = psm.tile([P, gc], f32, tag="pr")
            for w in range(12):
                nc.tensor.matmul(
                    wu[:], lhsT=alt_sb[:, 0:P], rhs=alt_sb[:, 0:gc],
                    start=(w == 0), stop=(w == 11),
                )

            coef_sb = {k: cpool.tile([P, 2, QU], f32r, name=f"coef_{k}") for k in coef_r}
            dat0 = {k: wpool.tile([P, 2, gc], f32r, tag=k, name=f"dat0_{k}") for k in data_r}
            # loads ordered so the first (even-real) matmuls start early
            for j in range(2):
                nc.sync.dma_start(out=coef_sb["cer"][:, j], in_=coef_r["cer"][:, j])
                nc.sync.dma_start(out=dat0["aue"][:, j], in_=data_r["aue"][0][:, j])
            for j in range(2):
                nc.sync.dma_start(out=dat0["avo"][:, j], in_=data_r["avo"][0][:, j])
                nc.sync.dma_start(out=coef_sb["cei"][:, j], in_=coef_r["cei"][:, j])
            for j in range(2):
                nc.sync.dma_start(out=coef_sb["cor"][:, j], in_=coef_r["cor"][:, j])
                nc.sync.dma_start(out=dat0["auo"][:, j], in_=data_r["auo"][0][:, j])
            for j in range(2):
                nc.sync.dma_start(out=coef_sb["coi"][:, j], in_=coef_r["coi"][:, j])
                nc.sync.dma_start(out=dat0["ave"][:, j], in_=data_r["ave"][0][:, j])

            for g in range(n_groups):
                if g == 0:
                    dat = dat0
                else:
                    dat = {k: wpool.tile([P, 2, gc], f32r, tag=k, name=f"dat_{k}_{g}") for k in data_r}
                    for k in ("aue", "avo", "auo", "ave"):
                        nc.sync.dma_start(out=dat[k][:], in_=data_r[k][g])

                ortg = opool.tile([P, 4, gc], f32, tag="ortg")
                oitg = opool.tile([P, 4, gc], f32, tag="oitg")

                for kt in range(n_kt):
                    ksl = slice(kt * P, (kt + 1) * P)
                    # even-real: Aue @ Cer + U[256]*(-1)^m (via one-hot alt)
                    per = psm.tile([P, gc], f32, tag="pr")
                    for j in range(2):
                        nc.tensor.matmul(
                            per[:], lhsT=coef_sb["cer"][:, j, ksl],
                            rhs=dat["aue"][:, j], start=(j == 0), stop=False,
                        )
                    nc.tensor.matmul(
                        per[:], lhsT=alt_sb[:, 0:P], rhs=dat["avo"][:, 0],
                        start=False, stop=True,
                    )
                    # even-imag: Avo @ Cei (row 0 of Cei kills the U[256] slot)
                    pei = psm.tile([P, gc], f32, tag="pi")
                    for j in range(2):
                        nc.tensor.matmul(
                            pei[:], lhsT=coef_sb["cei"][:, j, ksl],
                            rhs=dat["avo"][:, j], start=(j == 0), stop=(j == 1),
                        )
                    # odd-real: Auo @ Cor (row 0 = ones absorbs U0 - x512)
                    por = psm.tile([P, gc], f32, tag="pr")
                    for j in range(2):
                        nc.tensor.matmul(
                            por[:], lhsT=coef_sb["cor"][:, j, ksl],
                            rhs=dat["auo"][:, j], start=(j == 0), stop=(j == 1),
                        )
                    # odd-imag: Ave @ Coi (row 0 overridden, carries V[256])
                    poi = psm.tile([P, gc], f32, tag="pi")
                    for j in range(2):
                        nc.tensor.matmul(
                            poi[:], lhsT=coef_sb["coi"][:, j, ksl],
                            rhs=dat["ave"][:, j], start=(j == 0), stop=(j == 1),
                        )

                    nc.vector.tensor_copy(out=ortg[:, 2 * kt], in_=per[:])
                    nc.vector.tensor_copy(out=ortg[:, 2 * kt + 1], in_=por[:])
                    nc.scalar.copy(out=oitg[:, 2 * kt], in_=pei[:])
                    nc.scalar.copy(out=oitg[:, 2 * kt + 1], in_=poi[:])

                    # drain per k-tile: adjacent slots -> 4KB contiguous runs,
                    # and the final transfer shrinks to 0.5MB per tensor
                    ksl2 = slice(2 * kt, 2 * kt + 2)
                    nc.gpsimd.dma_start(out=ort_r[g][:, ksl2], in_=ortg[:, ksl2])
                    nc.scalar.dma_start(out=oit_r[g][:, ksl2], in_=oitg[:, ksl2])

    nc.compile()
    return nc


def _get_nc(b_core=B_CORE):
    if b_core not in _BUILD_CACHE:
        _BUILD_CACHE[b_core] = build_nc(b_core)
    return _BUILD_CACHE[b_core]


def _host_prep(x):
    """Two-level real-FFT folds (transposed) + host-side k=0 column."""
    B = x.shape[0]
    U = np.empty((B, HALF), dtype=np.float32)
    V = np.empty((B, HALF), dtype=np.float32)
    U[:, 0] = x[:, 0]
    rev = x[:, 1023:HALF:-1]
    np.add(x[:, 1:HALF], rev, out=U[:, 1:HALF])
    np.subtract(x[:, 1:HALF], rev, out=V[:, 1:HALF])
    x512 = x[:, HALF]
    a = {k: np.empty((B, QU), dtype=np.float32)
         for k in ("aue", "auo", "avo", "ave")}
    a["aue"][:, 0] = U[:, 0] + x512
    a["auo"][:, 0] = U[:, 0] - x512
    a["avo"][:, 0] = U[:, QU]                  # = x[256] + x[768]
    a["ave"][:, 0] = V[:, QU]                  # = x[256] - x[768]
    urev = U[:, 511:QU:-1]
    vrev = V[:, 511:QU:-1]
    np.add(U[:, 1:QU], urev, out=a["aue"][:, 1:QU])
    np.subtract(U[:, 1:QU], urev, out=a["auo"][:, 1:QU])
    np.subtract(V[:, 1:QU], vrev, out=a["avo"][:, 1:QU])
    np.add(V[:, 1:QU], vrev, out=a["ave"][:, 1:QU])
    col0 = (U.sum(axis=1, dtype=np.float64) + x512).astype(np.float32)
    at = {k: np.ascontiguousarray(v.T) for k, v in a.items()}   # [256, B]
    return at, col0


def _blocked(a_t, sl, b_core):
    """[256, B] column-slice -> group-blocked [n_groups, 256, gc] contiguous."""
    gc = min(GC, b_core)
    n_groups = b_core // gc
    s = a_t[:, sl]
    return np.ascontiguousarray(s.reshape(QU, n_groups, gc).transpose(1, 0, 2))


def _assemble(half_t, out, sl, b_core, neg_mirror):
    """Device half [n_groups, 512(r=4p+slot), gc] -> out[sl, :] (1024 cols).

    slot 0,1: even k = 2*(kt*128 + p + 1); slot 2,3: odd k = 2*(kt*128+p)+1.
    """
    gc = min(GC, b_core)
    n_groups = b_core // gc
    h = half_t.reshape(n_groups, P, 4, gc)
    b0 = sl.start
    for g in range(n_groups):
        rows = slice(b0 + g * gc, b0 + (g + 1) * gc)
        for kt in range(2):
            e0 = 2 * (kt * P) + 2
            out[rows, e0 : e0 + 2 * P : 2] = h[g, :, 2 * kt, :].T
            o0 = 2 * (kt * P) + 1
            out[rows, o0 : o0 + 2 * P : 2] = h[g, :, 2 * kt + 1, :].T
    blk = out[sl]
    if neg_mirror:
        np.negative(blk[:, 511:0:-1], out=blk[:, 513:1024])
    else:
        blk[:, 513:1024] = blk[:, 511:0:-1]


def kernel(**inputs):
    from concourse.bass_utils import run_bass_kernel_spmd

    x = np.ascontiguousarray(np.asarray(inputs["x"], dtype=np.float32))
    assert x.shape == (BATCH, N_FFT), x.shape
    cer, cei, cor, coi, alt = _constants()
    at, col0 = _host_prep(x)
    nc = _get_nc()
    in_maps = []
    for c in range(N_CORES):
        sl = slice(c * B_CORE, (c + 1) * B_CORE)
        m = {k: _blocked(v, sl, B_CORE) for k, v in at.items()}
        m.update({"cer": cer, "cei": cei, "cor": cor, "coi": coi, "alt": alt})
        in_maps.append(m)
    trace = bool(int(os.environ.get("FFT_KERNEL_TRACE", "0")))
    try:
        res = run_bass_kernel_spmd(
            nc, in_maps, core_ids=list(range(N_CORES)), trace=trace
        )
    except Exception:
        # transient NRT/device hiccups have been observed; retry once
        res = run_bass_kernel_spmd(
            nc, in_maps, core_ids=list(range(N_CORES)), trace=trace
        )
    if trace:
        kernel.last_results = res
    real = np.empty((BATCH, N_FFT), dtype=np.float32)
    imag = np.empty((BATCH, N_FFT), dtype=np.float32)
    for c in range(N_CORES):
        sl = slice(c * B_CORE, (c + 1) * B_CORE)
        _assemble(res.results[c]["o_rt"], real, sl, B_CORE, neg_mirror=False)
        _assemble(res.results[c]["o_it"], imag, sl, B_CORE, neg_mirror=True)
    real[:, 0] = col0
    imag[:, 0] = 0.0
    return real, imag



# revision 4
# speedup vs baseline: 1.3875x; 1.3875x over previous
"""Trainium2 Bass kernel: 1024-point FFT of real rows -> (real, imag).

Math: out = FFT_1024(x[b, :]) per row. Two folding levels over the real
input x (U[n] = x[n]+x[1024-n], V[n] = x[n]-x[1024-n]), then a radix-2
split of the half-spectrum k in [1,513) by parity:
  Xr[2m]   = sum_{n<256} Aue[n] cos(2pi n m/512)   + U[256](-1)^m
  Xi[2m]   = sum_{n<256} Avo[n] (-sin(2pi n m/512))
  Xr[2m+1] = sum_{n<256} Auo[n] cos(pi n(2m+1)/512)
  Xi[2m+1] = sum_{n<256} Ave[n] (-sin(pi n(2m+1)/512)) - V[256](-1)^m
with Aue/Auo/Avo/Ave the second-level even/odd folds of U and V. The
remaining half follows from X[1024-k] = conj(X[k]); k=0 is a row sum.

The device computes the four quadrants in TRANSPOSED orientation (freq
on PSUM partitions, batch on the free dim). All device I/O is fp16 —
the kernel is DMA-bandwidth-bound (16 DMA engines x ~22.5 B/ns), so
halving the bytes halves the streaming time; fp16 quantization of the
host-folded data + coefficients keeps L2 relative error ~1e-3. Folded
arrays are laid out [p, j, g, b] so every DMA moves one 8KB contiguous
run per partition (4KB descriptors saturate the engines). The four
quarter-size cos/sin matrices ship as one packed [128,4,2,256] tensor
(4KB/partition, one DMA); the one-hot `alt` row used for the U[256]
rank-1 term is synthesized on-device with three memsets instead of
being DMA'd. Outputs drain per k-tile as fp16 (4KB runs/partition) on
the gpsimd/scalar queues while inputs ride the sync queue.

Per 1024-row group and 128-wide k-tile: fp16 matmuls with K=256 (2
accumulating chunks) per 512-col PSUM half; every rank-1 edge term
rides a dead coefficient row: row 0 of Aue/Auo absorbs U[0] +/- x[512]
(all-ones coefficient rows), row 0 of Avo carries U[256] (killed by
Cei's zero row, applied to the even-real quadrant by one extra matmul
against the one-hot `alt` row), and row 0 of Ave carries V[256] via
Coi's overridden row 0.

The host performs the pure data-expansion assembly: parity interleave,
conjugate mirror, k=0 column, final layout transpose, fp16->fp32
upcast. Pure data-parallel across 8 cores, no collectives.
"""

import os
import numpy as np

N_FFT = 1024
BATCH = 16384
N_CORES = 8
B_CORE = BATCH // N_CORES  # 2048
P = 128
HALF = 512
QU = 256
GC = 1024                  # batch rows per group

_BUILD_CACHE = {}


def _constants():
    n = np.arange(QU, dtype=np.float64)[:, None]
    c = np.arange(QU, dtype=np.float64)[None, :]
    cer = np.cos(2 * np.pi * n * (c + 1) / 512)       # row 0 = 1 (absorbs U0+x512)
    cei = -np.sin(2 * np.pi * n * (c + 1) / 512)      # row 0 = 0 (kills U[256] slot)
    cor = np.cos(np.pi * n * (2 * c + 1) / 512)       # row 0 = 1 (absorbs U0-x512)
    coi = -np.sin(np.pi * n * (2 * c + 1) / 512)
    coi[0, :] = np.where(np.arange(QU) % 2 == 0, -1.0, 1.0)  # carries -V[256](-1)^m
    # packed [p, matrix, j, k] with row n = 2p+j; fp16, 4KB per partition
    packed = np.stack(
        [m.reshape(P, 2, QU) for m in (cer, cei, cor, coi)], axis=1
    ).astype(np.float16)
    return np.ascontiguousarray(packed)


def build_nc(b_core=B_CORE):
    """Build + compile the per-core Bass program (same NEFF on all cores)."""
    import concourse.mybir as mybir
    import concourse.tile as tile
    from concourse import bacc

    f16 = mybir.dt.float16
    f32 = mybir.dt.float32

    gc = min(GC, b_core)
    n_groups = b_core // gc
    n_kt = QU // P             # 128-wide k-tiles per quadrant (2)
    n_h = gc // HALF           # 512-col PSUM halves per group (2)

    nc = bacc.Bacc(
        "TRN2", target_bir_lowering=False, debug=False, num_devices=N_CORES
    )

    names = ("aue", "avo", "auo", "ave")   # order = DMA arrival order
    data_in = {
        name: nc.dram_tensor(name, [P, 2, n_groups, gc], f16, kind="ExternalInput")
        for name in names
    }
    coef_in = nc.dram_tensor("coef", [P, 4, 2, QU], f16, kind="ExternalInput")
    # transposed halves, group-blocked: row r = 4p + slot;
    # slots 0,1 = even-k tiles, 2,3 = odd-k tiles (host interleaves)
    o_rt = nc.dram_tensor("o_rt", [n_groups, 2 * QU, gc], f16, kind="ExternalOutput")
    o_it = nc.dram_tensor("o_it", [n_groups, 2 * QU, gc], f16, kind="ExternalOutput")

    ort_r = o_rt.ap().rearrange("g (p t) b -> g p t b", t=4)
    oit_r = o_it.ap().rearrange("g (p t) b -> g p t b", t=4)

    with tile.TileContext(nc) as tc:
        with (
            tc.tile_pool(name="const", bufs=1) as cpool,
            tc.tile_pool(name="work", bufs=1) as wpool,
            tc.tile_pool(name="outp", bufs=2) as opool,
            tc.tile_pool(name="psm", bufs=1, space="PSUM") as psm,
        ):
            # one-hot +/-1 row for the U[256] rank-1 term, synthesized on
            # device (saves a DMA); also the warmup matmul operand
            alt_sb = cpool.tile([P, HALF], f16)
            nc.gpsimd.memset(alt_sb[:], 0.0)
            nc.gpsimd.memset(alt_sb[0:1, 0:HALF:2], -1.0)
            nc.gpsimd.memset(alt_sb[0:1, 1:HALF:2], 1.0)

            coef_sb = cpool.tile([P, 4, 2, QU], f16)
            nc.sync.dma_start(out=coef_sb[:], in_=coef_in.ap())
            dat = {k: wpool.tile([P, 2, n_groups, gc], f16, tag=k, name=f"dat_{k}")
                   for k in names}
            for k in names:
                nc.sync.dma_start(out=dat[k][:], in_=data_in[k].ap())

            # warmup: keep the PE busy ramping its clock while inputs stream
            # in (borrows the "poi" psum slot; poi is the last real user)
            wu = psm.tile([P, gc], f32, tag="poi")
            for w in range(12):
                nc.tensor.matmul(
                    wu[:, 0:HALF], lhsT=alt_sb[:, 0:P], rhs=alt_sb[:, 0:HALF],
                    start=(w == 0), stop=(w == 11),
                )

            for g in range(n_groups):
                ortg = opool.tile([P, 4, gc], f16, tag="ortg")
                oitg = opool.tile([P, 4, gc], f16, tag="oitg")

                for kt in range(n_kt):
                    ksl = slice(kt * P, (kt + 1) * P)
                    # even-real: Aue @ Cer + U[256]*(-1)^m (via one-hot alt)
                    per = psm.tile([P, gc], f32, tag="per")
                    for h in range(n_h):
                        bsl = slice(h * HALF, (h + 1) * HALF)
                        for j in range(2):
                            nc.tensor.matmul(
                                per[:, bsl], lhsT=coef_sb[:, 0, j, ksl],
                                rhs=dat["aue"][:, j, g, bsl],
                                start=(j == 0), stop=False,
                            )
                        nc.tensor.matmul(
                            per[:, bsl], lhsT=alt_sb[:, 0:P],
                            rhs=dat["avo"][:, 0, g, bsl],
                            start=False, stop=True,
                        )
                    # even-imag: Avo @ Cei (row 0 of Cei kills the U[256] slot)
                    pei = psm.tile([P, gc], f32, tag="pei")
                    for h in range(n_h):
                        bsl = slice(h * HALF, (h + 1) * HALF)
                        for j in range(2):
                            nc.tensor.matmul(
                                pei[:, bsl], lhsT=coef_sb[:, 1, j, ksl],
                                rhs=dat["avo"][:, j, g, bsl],
                                start=(j == 0), stop=(j == 1),
                            )
                    # odd-real: Auo @ Cor (row 0 = ones absorbs U0 - x512)
                    por = psm.tile([P, gc], f32, tag="por")
                    for h in range(n_h):
                        bsl = slice(h * HALF, (h + 1) * HALF)
                        for j in range(2):
                            nc.tensor.matmul(
                                por[:, bsl], lhsT=coef_sb[:, 2, j, ksl],
                                rhs=dat["auo"][:, j, g, bsl],
                                start=(j == 0), stop=(j == 1),
                            )
                    # odd-imag: Ave @ Coi (row 0 overridden, carries V[256])
                    poi = psm.tile([P, gc], f32, tag="poi")
                    for h in range(n_h):
                        bsl = slice(h * HALF, (h + 1) * HALF)
                        for j in range(2):
                            nc.tensor.matmul(
                                poi[:, bsl], lhsT=coef_sb[:, 3, j, ksl],
                                rhs=dat["ave"][:, j, g, bsl],
                                start=(j == 0), stop=(j == 1),
                            )

                    # PSUM f32 -> SBUF f16 staging (gpsimd cannot read PSUM)
                    nc.vector.tensor_copy(out=ortg[:, 2 * kt], in_=per[:])
                    nc.vector.tensor_copy(out=ortg[:, 2 * kt + 1], in_=por[:])
                    nc.scalar.copy(out=oitg[:, 2 * kt], in_=pei[:])
                    nc.scalar.copy(out=oitg[:, 2 * kt + 1], in_=poi[:])

                    # drain per k-tile: adjacent slots -> 4KB contiguous runs
                    ksl2 = slice(2 * kt, 2 * kt + 2)
                    nc.gpsimd.dma_start(out=ort_r[g][:, ksl2], in_=ortg[:, ksl2])
                    nc.gpsimd.dma_start(out=oit_r[g][:, ksl2], in_=oitg[:, ksl2])

    nc.compile()
    return nc


def _get_nc(b_core=B_CORE):
    if b_core not in _BUILD_CACHE:
        _BUILD_CACHE[b_core] = build_nc(b_core)
    return _BUILD_CACHE[b_core]


def _host_prep(x):
    """Two-level real-FFT folds (transposed) + host-side k=0 column."""
    B = x.shape[0]
    U = np.empty((B, HALF), dtype=np.float32)
    V = np.empty((B, HALF), dtype=np.float32)
    U[:, 0] = x[:, 0]
    rev = x[:, 1023:HALF:-1]
    np.add(x[:, 1:HALF], rev, out=U[:, 1:HALF])
    np.subtract(x[:, 1:HALF], rev, out=V[:, 1:HALF])
    x512 = x[:, HALF]
    a = {k: np.empty((B, QU), dtype=np.float32)
         for k in ("aue", "auo", "avo", "ave")}
    a["aue"][:, 0] = U[:, 0] + x512
    a["auo"][:, 0] = U[:, 0] - x512
    a["avo"][:, 0] = U[:, QU]                  # = x[256] + x[768]
    a["ave"][:, 0] = V[:, QU]                  # = x[256] - x[768]
    urev = U[:, 511:QU:-1]
    vrev = V[:, 511:QU:-1]
    np.add(U[:, 1:QU], urev, out=a["aue"][:, 1:QU])
    np.subtract(U[:, 1:QU], urev, out=a["auo"][:, 1:QU])
    np.subtract(V[:, 1:QU], vrev, out=a["avo"][:, 1:QU])
    np.add(V[:, 1:QU], vrev, out=a["ave"][:, 1:QU])
    col0 = (U.sum(axis=1, dtype=np.float64) + x512).astype(np.float32)
    at = {k: np.ascontiguousarray(v.T, dtype=np.float16)   # [256, B] fp16
          for k, v in a.items()}
    return at, col0


def _blocked(a_t, sl, b_core):
    """[256, B] fp16 column-slice -> [128(p), 2(j), n_groups, gc]."""
    gc = min(GC, b_core)
    n_groups = b_core // gc
    s = np.ascontiguousarray(a_t[:, sl])
    return s.reshape(P, 2, n_groups, gc)


def _assemble(half_t, out, sl, b_core, neg_mirror):
    """Device half [n_groups, 512(r=4p+slot), gc] -> out[sl, :] (1024 cols).

    slot 0,1: even k = 2*(kt*128 + p + 1); slot 2,3: odd k = 2*(kt*128+p)+1.
    """
    gc = min(GC, b_core)
    n_groups = b_core // gc
    h = half_t.reshape(n_groups, P, 4, gc)
    b0 = sl.start
    for g in range(n_groups):
        rows = slice(b0 + g * gc, b0 + (g + 1) * gc)
        for kt in range(2):
            e0 = 2 * (kt * P) + 2
            out[rows, e0 : e0 + 2 * P : 2] = h[g, :, 2 * kt, :].T
            o0 = 2 * (kt * P) + 1
            out[rows, o0 : o0 + 2 * P : 2] = h[g, :, 2 * kt + 1, :].T
    blk = out[sl]
    if neg_mirror:
        np.negative(blk[:, 511:0:-1], out=blk[:, 513:1024])
    else:
        blk[:, 513:1024] = blk[:, 511:0:-1]


def kernel(**inputs):
    from concourse.bass_utils import run_bass_kernel_spmd

    x = np.ascontiguousarray(np.asarray(inputs["x"], dtype=np.float32))
    assert x.shape == (BATCH, N_FFT), x.shape
    coef = _constants()
    at, col0 = _host_prep(x)
    nc = _get_nc()
    in_maps = []
    for c in range(N_CORES):
        sl = slice(c * B_CORE, (c + 1) * B_CORE)
        m = {k: _blocked(v, sl, B_CORE) for k, v in at.items()}
        m["coef"] = coef
        in_maps.append(m)
    trace = bool(int(os.environ.get("FFT_KERNEL_TRACE", "0")))
    try:
        res = run_bass_kernel_spmd(
            nc, in_maps, core_ids=list(range(N_CORES)), trace=trace
        )
    except Exception:
        # transient NRT/device hiccups have been observed; retry once
        res = run_bass_kernel_spmd(
            nc, in_maps, core_ids=list(range(N_CORES)), trace=trace
        )
    if trace:
        kernel.last_results = res
    real = np.empty((BATCH, N_FFT), dtype=np.float32)
    imag = np.empty((BATCH, N_FFT), dtype=np.float32)
    for c in range(N_CORES):
        sl = slice(c * B_CORE, (c + 1) * B_CORE)
        _assemble(res.results[c]["o_rt"], real, sl, B_CORE, neg_mirror=False)
        _assemble(res.results[c]["o_it"], imag, sl, B_CORE, neg_mirror=True)
    real[:, 0] = col0
    imag[:, 0] = 0.0
    return real, imag


# revision 9
# speedup vs baseline: 1.5327x; 1.1046x over previous
"""Trainium2 Bass kernel: 1024-point FFT of real rows -> (real, imag).

Math: out = FFT_1024(x[b, :]) per row. Two folding levels over the real
input x (U[n] = x[n]+x[1024-n], V[n] = x[n]-x[1024-n]), then a radix-2
split of the half-spectrum k in [1,513) by parity:
  Xr[2m]   = sum_{n<256} Aue[n] cos(2pi n m/512)   + U[256](-1)^m
  Xi[2m]   = sum_{n<256} Avo[n] (-sin(2pi n m/512))
  Xr[2m+1] = sum_{n<256} Auo[n] cos(pi n(2m+1)/512)
  Xi[2m+1] = sum_{n<256} Ave[n] (-sin(pi n(2m+1)/512)) - V[256](-1)^m
with Aue/Auo/Avo/Ave the second-level even/odd folds of U and V. The
remaining half follows from X[1024-k] = conj(X[k]); k=0 is a row sum.

The device computes the four quadrants in TRANSPOSED orientation (freq
on PSUM partitions, batch on the free dim). All device I/O is fp16 —
the kernel is DMA-bandwidth-bound (16 DMA engines x ~22.5 B/ns), so
halving the bytes halves the streaming time; fp16 quantization of the
host-folded data + coefficients keeps L2 relative error ~1e-3. Folded
arrays are laid out [p, j, g, b] so every DMA moves one 8KB contiguous
run per partition (4KB descriptors saturate the engines). The four
quarter-size cos/sin matrices ship as one packed [128,4,2,256] tensor
(4KB/partition, one DMA); the one-hot `alt` row used for the U[256]
rank-1 term is synthesized on-device with three memsets instead of
being DMA'd. Outputs drain per k-tile as fp16 (4KB runs/partition) on
the gpsimd/scalar queues while inputs ride the sync queue.

Per 1024-row group and 128-wide k-tile: fp16 matmuls with K=256 (2
accumulating chunks) per 512-col PSUM half; every rank-1 edge term
rides a dead coefficient row: row 0 of Aue/Auo absorbs U[0] +/- x[512]
(all-ones coefficient rows), row 0 of Avo carries U[256] (killed by
Cei's zero row, applied to the even-real quadrant by one extra matmul
against the one-hot `alt` row), and row 0 of Ave carries V[256] via
Coi's overridden row 0.

The host performs the pure data-expansion assembly: parity interleave,
conjugate mirror, k=0 column, final layout transpose, fp16->fp32
upcast. Pure data-parallel across 8 cores, no collectives.
"""

import os
import numpy as np

N_FFT = 1024
BATCH = 16384
N_CORES = 8
B_CORE = BATCH // N_CORES  # 2048
P = 128
HALF = 512
QU = 256
GC = 1024                  # batch rows per group

_BUILD_CACHE = {}


def _constants():
    n = np.arange(QU, dtype=np.float64)[:, None]
    c = np.arange(QU, dtype=np.float64)[None, :]
    cer = np.cos(2 * np.pi * n * (c + 1) / 512)       # row 0 = 1 (absorbs U0+x512)
    cei = -np.sin(2 * np.pi * n * (c + 1) / 512)      # row 0 = 0 (kills U[256] slot)
    cor = np.cos(np.pi * n * (2 * c + 1) / 512)       # row 0 = 1 (absorbs U0-x512)
    coi = -np.sin(np.pi * n * (2 * c + 1) / 512)
    coi[0, :] = np.where(np.arange(QU) % 2 == 0, -1.0, 1.0)  # carries -V[256](-1)^m
    # packed [p, matrix, j, k] with row n = 2p+j; fp16, 4KB per partition
    packed = np.stack(
        [m.reshape(P, 2, QU) for m in (cer, cei, cor, coi)], axis=1
    ).astype(np.float16)
    return np.ascontiguousarray(packed)


def build_nc(b_core=B_CORE):
    """Build + compile the per-core Bass program (same NEFF on all cores)."""
    import concourse.mybir as mybir
    import concourse.tile as tile
    from concourse import bacc

    f16 = mybir.dt.float16
    f32 = mybir.dt.float32

    gc = min(GC, b_core)
    n_groups = b_core // gc
    n_kt = QU // P             # 128-wide k-tiles per quadrant (2)
    n_h = gc // HALF           # 512-col PSUM halves per group (2)

    nc = bacc.Bacc(
        "TRN2", target_bir_lowering=False, debug=False, num_devices=N_CORES
    )

    names = ("aue", "avo", "auo", "ave")   # order = DMA arrival order
    data_in = {
        name: nc.dram_tensor(name, [P, n_groups, 2, gc], f16, kind="ExternalInput")
        for name in names
    }
    coef_in = nc.dram_tensor("coef", [P, 4, 2, QU], f16, kind="ExternalInput")
    # transposed halves, group-blocked: row r = 4p + slot;
    # slots 0,1 = even-k tiles, 2,3 = odd-k tiles (host interleaves)
    o_rt = nc.dram_tensor("o_rt", [n_groups, 2 * QU, gc], f16, kind="ExternalOutput")
    o_it = nc.dram_tensor("o_it", [n_groups, 2 * QU, gc], f16, kind="ExternalOutput")

    ort_r = o_rt.ap().rearrange("g (p t) b -> g p t b", t=4)
    oit_r = o_it.ap().rearrange("g (p t) b -> g p t b", t=4)

    with tile.TileContext(nc) as tc:
        with (
            tc.tile_pool(name="const", bufs=1) as cpool,
            tc.tile_pool(name="work", bufs=1) as wpool,
            tc.tile_pool(name="outp", bufs=2) as opool,
            tc.tile_pool(name="psm", bufs=1, space="PSUM") as psm,
        ):
            # one-hot +/-1 row for the U[256] rank-1 term, synthesized on
            # device (saves a DMA); also the warmup matmul operand
            alt_sb = cpool.tile([P, HALF], f16)
            nc.gpsimd.memset(alt_sb[:], 0.0)
            nc.gpsimd.memset(alt_sb[0:1, 0:HALF:2], -1.0)
            nc.gpsimd.memset(alt_sb[0:1, 1:HALF:2], 1.0)

            coef_sb = cpool.tile([P, 4, 2, QU], f16)
            nc.sync.dma_start(out=coef_sb[:], in_=coef_in.ap())
            # per-(group, array) 512KB chunks: group 0's arrays arrive first
            # so its compute + output drains overlap group 1's input stream
            dat = {k: wpool.tile([P, n_groups, 2, gc], f16, tag=k, name=f"dat_{k}")
                   for k in names}
            for g in range(n_groups):
                for k in names:
                    nc.sync.dma_start(out=dat[k][:, g], in_=data_in[k].ap()[:, g])

            # warmup: keep the PE busy ramping its clock until the first real
            # operands land (borrows a "B" psum slot; EI is the next user)
            wu = psm.tile([P, gc], f32, tag="B", bufs=2)
            for w in range(8):
                nc.tensor.matmul(
                    wu[:, 0:HALF], lhsT=alt_sb[:, 0:P], rhs=alt_sb[:, 0:HALF],
                    start=(w == 0), stop=(w == 7),
                )

            # quadrant matmul emitter: 2 accumulating K-chunks per 512-col
            # PSUM half, plus the rank-1 `alt` term for the even-real quadrant
            def quad(ps, mi, dk, g, ksl, with_alt):
                for h in range(n_h):
                    bsl = slice(h * HALF, (h + 1) * HALF)
                    for j in range(2):
                        nc.tensor.matmul(
                            ps[:, bsl], lhsT=coef_sb[:, mi, j, ksl],
                            rhs=dat[dk][:, g, j, bsl],
                            start=(j == 0), stop=(j == 1) and not with_alt,
                        )
                    if with_alt:
                        nc.tensor.matmul(
                            ps[:, bsl], lhsT=alt_sb[:, 0:P],
                            rhs=dat["avo"][:, g, 0, bsl],
                            start=False, stop=True,
                        )

            for g in range(n_groups):
                ortg = opool.tile([P, 4, gc], f16, tag="ortg")
                oitg = opool.tile([P, 4, gc], f16, tag="oitg")

                # quadrant-major order matches input arrival
                # (aue -> avo -> auo -> ave); PSUM: 2 tags x 2 bufs x 2 banks
                for kt in range(n_kt):
                    ksl = slice(kt * P, (kt + 1) * P)
                    # even-real: Aue @ Cer + U[256]*(-1)^m (via one-hot alt)
                    per = psm.tile([P, gc], f32, tag="A", bufs=2)
                    quad(per, 0, "aue", g, ksl, with_alt=True)
                    nc.vector.tensor_copy(out=ortg[:, 2 * kt], in_=per[:])
                for kt in range(n_kt):
                    ksl = slice(kt * P, (kt + 1) * P)
                    # even-imag: Avo @ Cei (row 0 of Cei kills U[256] slot)
                    pei = psm.tile([P, gc], f32, tag="B", bufs=2)
                    quad(pei, 1, "avo", g, ksl, with_alt=False)
                    nc.scalar.copy(out=oitg[:, 2 * kt], in_=pei[:])
                for kt in range(n_kt):
                    ksl = slice(kt * P, (kt + 1) * P)
                    # odd-real: Auo @ Cor (row 0 = ones absorbs U0 - x512)
                    por = psm.tile([P, gc], f32, tag="A", bufs=2)
                    quad(por, 2, "auo", g, ksl, with_alt=False)
                    nc.vector.tensor_copy(out=ortg[:, 2 * kt + 1], in_=por[:])
                    # o_rt k-tile complete (slots 2kt, 2kt+1): drain as 4KB runs
                    ksl2 = slice(2 * kt, 2 * kt + 2)
                    nc.gpsimd.dma_start(out=ort_r[g][:, ksl2], in_=ortg[:, ksl2])
                for kt in range(n_kt):
                    ksl = slice(kt * P, (kt + 1) * P)
                    # odd-imag: Ave @ Coi (row 0 overridden, carries V[256])
                    poi = psm.tile([P, gc], f32, tag="B", bufs=2)
                    quad(poi, 3, "ave", g, ksl, with_alt=False)
                    nc.scalar.copy(out=oitg[:, 2 * kt + 1], in_=poi[:])
                    ksl2 = slice(2 * kt, 2 * kt + 2)
                    nc.scalar.dma_start(out=oit_r[g][:, ksl2], in_=oitg[:, ksl2])

    nc.compile()
    return nc


def _get_nc(b_core=B_CORE):
    if b_core not in _BUILD_CACHE:
        _BUILD_CACHE[b_core] = build_nc(b_core)
    return _BUILD_CACHE[b_core]


def _host_prep(x):
    """Two-level real-FFT folds (transposed) + host-side k=0 column."""
    B = x.shape[0]
    U = np.empty((B, HALF), dtype=np.float32)
    V = np.empty((B, HALF), dtype=np.float32)
    U[:, 0] = x[:, 0]
    rev = x[:, 1023:HALF:-1]
    np.add(x[:, 1:HALF], rev, out=U[:, 1:HALF])
    np.subtract(x[:, 1:HALF], rev, out=V[:, 1:HALF])
    x512 = x[:, HALF]
    a = {k: np.empty((B, QU), dtype=np.float32)
         for k in ("aue", "auo", "avo", "ave")}
    a["aue"][:, 0] = U[:, 0] + x512
    a["auo"][:, 0] = U[:, 0] - x512
    a["avo"][:, 0] = U[:, QU]                  # = x[256] + x[768]
    a["ave"][:, 0] = V[:, QU]                  # = x[256] - x[768]
    urev = U[:, 511:QU:-1]
    vrev = V[:, 511:QU:-1]
    np.add(U[:, 1:QU], urev, out=a["aue"][:, 1:QU])
    np.subtract(U[:, 1:QU], urev, out=a["auo"][:, 1:QU])
    np.subtract(V[:, 1:QU], vrev, out=a["avo"][:, 1:QU])
    np.add(V[:, 1:QU], vrev, out=a["ave"][:, 1:QU])
    col0 = (U.sum(axis=1, dtype=np.float64) + x512).astype(np.float32)
    at = {k: np.ascontiguousarray(v.T, dtype=np.float16)   # [256, B] fp16
          for k, v in a.items()}
    return at, col0


def _blocked(a_t, sl, b_core):
    """[256, B] fp16 column-slice -> [128(p), n_groups, 2(j), gc]."""
    gc = min(GC, b_core)
    n_groups = b_core // gc
    s = a_t[:, sl].reshape(P, 2, n_groups, gc)          # [p, j, g, b]
    return np.ascontiguousarray(s.transpose(0, 2, 1, 3))  # [p, g, j, b]


def _assemble(half_t, out, sl, b_core, neg_mirror):
    """Device half [n_groups, 512(r=4p+slot), gc] -> out[sl, :] (1024 cols).

    slot 0,1: even k = 2*(kt*128 + p + 1); slot 2,3: odd k = 2*(kt*128+p)+1.
    """
    gc = min(GC, b_core)
    n_groups = b_core // gc
    h = half_t.reshape(n_groups, P, 4, gc)
    b0 = sl.start
    for g in range(n_groups):
        rows = slice(b0 + g * gc, b0 + (g + 1) * gc)
        for kt in range(2):
            e0 = 2 * (kt * P) + 2
            out[rows, e0 : e0 + 2 * P : 2] = h[g, :, 2 * kt, :].T
            o0 = 2 * (kt * P) + 1
            out[rows, o0 : o0 + 2 * P : 2] = h[g, :, 2 * kt + 1, :].T
    blk = out[sl]
    if neg_mirror:
        np.negative(blk[:, 511:0:-1], out=blk[:, 513:1024])
    else:
        blk[:, 513:1024] = blk[:, 511:0:-1]


def kernel(**inputs):
    from concourse.bass_utils import run_bass_kernel_spmd

    x = np.ascontiguousarray(np.asarray(inputs["x"], dtype=np.float32))
    assert x.shape == (BATCH, N_FFT), x.shape
    coef = _constants()
    at, col0 = _host_prep(x)
    nc = _get_nc()
    in_maps = []
    for c in range(N_CORES):
        sl = slice(c * B_CORE, (c + 1) * B_CORE)
        m = {k: _blocked(v, sl, B_CORE) for k, v in at.items()}
        m["coef"] = coef
        in_maps.append(m)
    trace = bool(int(os.environ.get("FFT_KERNEL_TRACE", "0")))
    try:
        res = run_bass_kernel_spmd(
            nc, in_maps, core_ids=list(range(N_CORES)), trace=trace
        )
    except Exception:
        # transient NRT/device hiccups have been observed; retry once
        res = run_bass_kernel_spmd(
            nc, in_maps, core_ids=list(range(N_CORES)), trace=trace
        )
    if trace:
        kernel.last_results = res
    real = np.empty((BATCH, N_FFT), dtype=np.float32)
    imag = np.empty((BATCH, N_FFT), dtype=np.float32)
    for c in range(N_CORES):
        sl = slice(c * B_CORE, (c + 1) * B_CORE)
        _assemble(res.results[c]["o_rt"], real, sl, B_CORE, neg_mirror=False)
        _assemble(res.results[c]["o_it"], imag, sl, B_CORE, neg_mirror=True)
    real[:, 0] = col0
    imag[:, 0] = 0.0
    return real, imag


# revision 13
# speedup vs baseline: 1.5466x; 1.0091x over previous
"""Trainium2 Bass kernel: 1024-point FFT of real rows -> (real, imag).

Math: out = FFT_1024(x[b, :]) per row. Two folding levels over the real
input x (U[n] = x[n]+x[1024-n], V[n] = x[n]-x[1024-n]), then a radix-2
split of the half-spectrum k in [1,513) by parity:
  Xr[2m]   = sum_{n<256} Aue[n] cos(2pi n m/512)   + U[256](-1)^m
  Xi[2m]   = sum_{n<256} Avo[n] (-sin(2pi n m/512))
  Xr[2m+1] = sum_{n<256} Auo[n] cos(pi n(2m+1)/512)
  Xi[2m+1] = sum_{n<256} Ave[n] (-sin(pi n(2m+1)/512)) - V[256](-1)^m
with Aue/Auo/Avo/Ave the second-level even/odd folds of U and V. The
remaining half follows from X[1024-k] = conj(X[k]); k=0 is a row sum.

The device computes the four quadrants in TRANSPOSED orientation (freq
on PSUM partitions, batch on the free dim). All device I/O is fp16 —
the kernel is DMA-bandwidth-bound (16 DMA engines x ~22.5 B/ns), so
halving the bytes halves the streaming time; fp16 quantization of the
host-folded data + coefficients keeps L2 relative error ~1e-3. Folded
arrays are laid out [p, j, g, b] so every DMA moves one 8KB contiguous
run per partition (4KB descriptors saturate the engines). The four
quarter-size cos/sin matrices ship as one packed [128,4,2,256] tensor
(4KB/partition, one DMA); the one-hot `alt` row used for the U[256]
rank-1 term is synthesized on-device with three memsets instead of
being DMA'd. Outputs drain per k-tile as fp16 (4KB runs/partition) on
the gpsimd/scalar queues while inputs ride the sync queue.

Per 1024-row group and 128-wide k-tile: fp16 matmuls with K=256 (2
accumulating chunks) per 512-col PSUM half; every rank-1 edge term
rides a dead coefficient row: row 0 of Aue/Auo absorbs U[0] +/- x[512]
(all-ones coefficient rows), row 0 of Avo carries U[256] (killed by
Cei's zero row, applied to the even-real quadrant by one extra matmul
against the one-hot `alt` row), and row 0 of Ave carries V[256] via
Coi's overridden row 0.

The host performs the pure data-expansion assembly: parity interleave,
conjugate mirror, k=0 column, final layout transpose, fp16->fp32
upcast. Pure data-parallel across 8 cores, no collectives.
"""

import os
import numpy as np

N_FFT = 1024
BATCH = 16384
N_CORES = 8
B_CORE = BATCH // N_CORES  # 2048
P = 128
HALF = 512
QU = 256
GC = 1024                  # batch rows per group

_BUILD_CACHE = {}


def _constants():
    n = np.arange(QU, dtype=np.float64)[:, None]
    c = np.arange(QU, dtype=np.float64)[None, :]
    cer = np.cos(2 * np.pi * n * (c + 1) / 512)       # row 0 = 1 (absorbs U0+x512)
    cei = -np.sin(2 * np.pi * n * (c + 1) / 512)      # row 0 = 0 (kills U[256] slot)
    cor = np.cos(np.pi * n * (2 * c + 1) / 512)       # row 0 = 1 (absorbs U0-x512)
    coi = -np.sin(np.pi * n * (2 * c + 1) / 512)
    coi[0, :] = np.where(np.arange(QU) % 2 == 0, -1.0, 1.0)  # carries -V[256](-1)^m
    # packed [p, matrix, j, k] with row n = 2p+j; fp16, 4KB per partition
    packed = np.stack(
        [m.reshape(P, 2, QU) for m in (cer, cei, cor, coi)], axis=1
    ).astype(np.float16)
    return np.ascontiguousarray(packed)


def build_nc(b_core=B_CORE):
    """Build + compile the per-core Bass program (same NEFF on all cores)."""
    import concourse.mybir as mybir
    import concourse.tile as tile
    from concourse import bacc

    f16 = mybir.dt.float16
    f32 = mybir.dt.float32

    gc = min(GC, b_core)
    n_groups = b_core // gc
    n_kt = QU // P             # 128-wide k-tiles per quadrant (2)
    n_h = gc // HALF           # 512-col PSUM halves per group (2)

    nc = bacc.Bacc(
        "TRN2", target_bir_lowering=False, debug=False, num_devices=N_CORES
    )

    names = ("aue", "avo", "auo", "ave")   # order = DMA arrival order
    data_in = {
        name: nc.dram_tensor(name, [P, n_groups, 2, gc], f16, kind="ExternalInput")
        for name in names
    }
    coef_in = nc.dram_tensor("coef", [P, 4, 2, QU], f16, kind="ExternalInput")
    # transposed halves, group-blocked: row r = 4p + slot;
    # slots 0,1 = even-k tiles, 2,3 = odd-k tiles (host interleaves)
    o_rt = nc.dram_tensor("o_rt", [n_groups, 2 * QU, gc], f16, kind="ExternalOutput")
    o_it = nc.dram_tensor("o_it", [n_groups, 2 * QU, gc], f16, kind="ExternalOutput")

    ort_r = o_rt.ap().rearrange("g (p t) b -> g p t b", t=4)
    oit_r = o_it.ap().rearrange("g (p t) b -> g p t b", t=4)

    with tile.TileContext(nc) as tc:
        with (
            tc.tile_pool(name="const", bufs=1) as cpool,
            tc.tile_pool(name="work", bufs=1) as wpool,
            tc.tile_pool(name="outp", bufs=2) as opool,
            tc.tile_pool(name="psm", bufs=1, space="PSUM") as psm,
        ):
            # one-hot +/-1 row for the U[256] rank-1 term, synthesized on
            # device (saves a DMA); also the warmup matmul operand
            alt_sb = cpool.tile([P, HALF], f16)
            nc.gpsimd.memset(alt_sb[:], 0.0)
            nc.gpsimd.memset(alt_sb[0:1, 0:HALF:2], -1.0)
            nc.gpsimd.memset(alt_sb[0:1, 1:HALF:2], 1.0)

            coef_sb = cpool.tile([P, 4, 2, QU], f16)
            nc.sync.dma_start(out=coef_sb[:], in_=coef_in.ap())
            # per-(group, array) 512KB chunks: group 0's arrays arrive first
            # so its compute + output drains overlap group 1's input stream
            dat = {k: wpool.tile([P, n_groups, 2, gc], f16, tag=k, name=f"dat_{k}")
                   for k in names}
            for g in range(n_groups):
                for k in names:
                    nc.sync.dma_start(out=dat[k][:, g], in_=data_in[k].ap()[:, g])

            # warmup: keep the PE busy ramping its clock until the first real
            # operands land (borrows a "B" psum slot; EI is the next user)
            wu = psm.tile([P, gc], f32, tag="B", bufs=2)
            for w in range(13):
                nc.tensor.matmul(
                    wu[:, 0:HALF], lhsT=alt_sb[:, 0:P], rhs=alt_sb[:, 0:HALF],
                    start=(w == 0), stop=(w == 12),
                )

            # quadrant matmul emitter: 2 accumulating K-chunks per 512-col
            # PSUM half, plus the rank-1 `alt` term for the even-real quadrant
            def quad(ps, mi, dk, g, ksl, with_alt):
                for h in range(n_h):
                    bsl = slice(h * HALF, (h + 1) * HALF)
                    for j in range(2):
                        nc.tensor.matmul(
                            ps[:, bsl], lhsT=coef_sb[:, mi, j, ksl],
                            rhs=dat[dk][:, g, j, bsl],
                            start=(j == 0), stop=(j == 1) and not with_alt,
                        )
                    if with_alt:
                        nc.tensor.matmul(
                            ps[:, bsl], lhsT=alt_sb[:, 0:P],
                            rhs=dat["avo"][:, g, 0, bsl],
                            start=False, stop=True,
                        )

            for g in range(n_groups):
                ortg = opool.tile([P, 4, gc], f16, tag="ortg")
                oitg = opool.tile([P, 4, gc], f16, tag="oitg")

                # quadrant-major order matches input arrival
                # (aue -> avo -> auo -> ave); PSUM: 2 tags x 2 bufs x 2 banks.
                # Output slots pair the two k-tiles of one quadrant (t=0,1:
                # even-k kt0/kt1; t=2,3: odd-k kt0/kt1) so each quadrant
                # drains as soon as its own copies land; kt0 copies ride
                # vector and kt1 scalar so every pair converts in parallel.
                def copy_out(stage, t, ps, kt):
                    if kt == 0:
                        nc.vector.tensor_copy(out=stage[:, t], in_=ps[:])
                    else:
                        nc.scalar.copy(out=stage[:, t], in_=ps[:])

                for kt in range(n_kt):
                    ksl = slice(kt * P, (kt + 1) * P)
                    # even-real: Aue @ Cer + U[256]*(-1)^m (via one-hot alt)
                    per = psm.tile([P, gc], f32, tag="A", bufs=2)
                    quad(per, 0, "aue", g, ksl, with_alt=True)
                    copy_out(ortg, kt, per, kt)
                nc.gpsimd.dma_start(out=ort_r[g][:, 0:2], in_=ortg[:, 0:2])
                for kt in range(n_kt):
                    ksl = slice(kt * P, (kt + 1) * P)
                    # even-imag: Avo @ Cei (row 0 of Cei kills U[256] slot)
                    pei = psm.tile([P, gc], f32, tag="B", bufs=2)
                    quad(pei, 1, "avo", g, ksl, with_alt=False)
                    copy_out(oitg, kt, pei, kt)
                nc.scalar.dma_start(out=oit_r[g][:, 0:2], in_=oitg[:, 0:2])
                for kt in range(n_kt):
                    ksl = slice(kt * P, (kt + 1) * P)
                    # odd-real: Auo @ Cor (row 0 = ones absorbs U0 - x512)
                    por = psm.tile([P, gc], f32, tag="A", bufs=2)
                    quad(por, 2, "auo", g, ksl, with_alt=False)
                    copy_out(ortg, 2 + kt, por, kt)
                nc.gpsimd.dma_start(out=ort_r[g][:, 2:4], in_=ortg[:, 2:4])
                for kt in range(n_kt):
                    ksl = slice(kt * P, (kt + 1) * P)
                    # odd-imag: Ave @ Coi (row 0 overridden, carries V[256])
                    poi = psm.tile([P, gc], f32, tag="B", bufs=2)
                    quad(poi, 3, "ave", g, ksl, with_alt=False)
                    copy_out(oitg, 2 + kt, poi, kt)
                nc.scalar.dma_start(out=oit_r[g][:, 2:4], in_=oitg[:, 2:4])

    nc.compile()
    return nc


def _get_nc(b_core=B_CORE):
    if b_core not in _BUILD_CACHE:
        _BUILD_CACHE[b_core] = build_nc(b_core)
    return _BUILD_CACHE[b_core]


def _host_prep(x):
    """Two-level real-FFT folds (transposed) + host-side k=0 column."""
    B = x.shape[0]
    U = np.empty((B, HALF), dtype=np.float32)
    V = np.empty((B, HALF), dtype=np.float32)
    U[:, 0] = x[:, 0]
    rev = x[:, 1023:HALF:-1]
    np.add(x[:, 1:HALF], rev, out=U[:, 1:HALF])
    np.subtract(x[:, 1:HALF], rev, out=V[:, 1:HALF])
    x512 = x[:, HALF]
    a = {k: np.empty((B, QU), dtype=np.float32)
         for k in ("aue", "auo", "avo", "ave")}
    a["aue"][:, 0] = U[:, 0] + x512
    a["auo"][:, 0] = U[:, 0] - x512
    a["avo"][:, 0] = U[:, QU]                  # = x[256] + x[768]
    a["ave"][:, 0] = V[:, QU]                  # = x[256] - x[768]
    urev = U[:, 511:QU:-1]
    vrev = V[:, 511:QU:-1]
    np.add(U[:, 1:QU], urev, out=a["aue"][:, 1:QU])
    np.subtract(U[:, 1:QU], urev, out=a["auo"][:, 1:QU])
    np.subtract(V[:, 1:QU], vrev, out=a["avo"][:, 1:QU])
    np.add(V[:, 1:QU], vrev, out=a["ave"][:, 1:QU])
    col0 = (U.sum(axis=1, dtype=np.float64) + x512).astype(np.float32)
    at = {k: np.ascontiguousarray(v.T, dtype=np.float16)   # [256, B] fp16
          for k, v in a.items()}
    return at, col0


def _blocked(a_t, sl, b_core):
    """[256, B] fp16 column-slice -> [128(p), n_groups, 2(j), gc]."""
    gc = min(GC, b_core)
    n_groups = b_core // gc
    s = a_t[:, sl].reshape(P, 2, n_groups, gc)          # [p, j, g, b]
    return np.ascontiguousarray(s.transpose(0, 2, 1, 3))  # [p, g, j, b]


def _assemble(half_t, out, sl, b_core, neg_mirror):
    """Device half [n_groups, 512(r=4p+slot), gc] -> out[sl, :] (1024 cols).

    slot 0,1: even k = 2*(kt*128 + p + 1), kt = slot;
    slot 2,3: odd  k = 2*(kt*128 + p) + 1, kt = slot - 2.
    """
    gc = min(GC, b_core)
    n_groups = b_core // gc
    h = half_t.reshape(n_groups, P, 4, gc)
    b0 = sl.start
    for g in range(n_groups):
        rows = slice(b0 + g * gc, b0 + (g + 1) * gc)
        for kt in range(2):
            e0 = 2 * (kt * P) + 2
            out[rows, e0 : e0 + 2 * P : 2] = h[g, :, kt, :].T
            o0 = 2 * (kt * P) + 1
            out[rows, o0 : o0 + 2 * P : 2] = h[g, :, 2 + kt, :].T
    blk = out[sl]
    if neg_mirror:
        np.negative(blk[:, 511:0:-1], out=blk[:, 513:1024])
    else:
        blk[:, 513:1024] = blk[:, 511:0:-1]


def kernel(**inputs):
    from concourse.bass_utils import run_bass_kernel_spmd

    x = np.ascontiguousarray(np.asarray(inputs["x"], dtype=np.float32))
    assert x.shape == (BATCH, N_FFT), x.shape
    coef = _constants()
    at, col0 = _host_prep(x)
    nc = _get_nc()
    in_maps = []
    for c in range(N_CORES):
        sl = slice(c * B_CORE, (c + 1) * B_CORE)
        m = {k: _blocked(v, sl, B_CORE) for k, v in at.items()}
        m["coef"] = coef
        in_maps.append(m)
    trace = bool(int(os.environ.get("FFT_KERNEL_TRACE", "0")))
    try:
        res = run_bass_kernel_spmd(
            nc, in_maps, core_ids=list(range(N_CORES)), trace=trace
        )
    except Exception:
        # transient NRT/device hiccups have been observed; retry once
        res = run_bass_kernel_spmd(
            nc, in_maps, core_ids=list(range(N_CORES)), trace=trace
        )
    if trace:
        kernel.last_results = res
    real = np.empty((BATCH, N_FFT), dtype=np.float32)
    imag = np.empty((BATCH, N_FFT), dtype=np.float32)
    for c in range(N_CORES):
        sl = slice(c * B_CORE, (c + 1) * B_CORE)
        _assemble(res.results[c]["o_rt"], real, sl, B_CORE, neg_mirror=False)
        _assemble(res.results[c]["o_it"], imag, sl, B_CORE, neg_mirror=True)
    real[:, 0] = col0
    imag[:, 0] = 0.0
    return real, imag
